# revision 5
# baseline (speedup 1.0000x reference)
"""DigiCaps routing kernel for Trainium2 (8 NeuronCores, data-parallel).

Math: the reference initializes routing logits bs = 0 and uses a single
shared Linear for every capsule, so softmax coefficients stay identical
across the K=10 capsules through all 3 routing iterations (verified: the
reference output has exactly zero spread over k).  The whole module
collapses to a per-batch recursion on raw u ([N=1152, 8]):

    r      = sum_n u[n, :]
    s0     = W @ (r / N) + b                  # [16]
    v      = squash_diag(s0);  V = v
    repeat twice:
        g  = W^T V                            # [8]
        c  = u @ g                            # [N]   (n-const terms drop in softmax)
        E  = exp(c);  Z = sum(E)              # no max-subtraction needed (|c| < 9)
        h  = u^T @ E                          # [8]
        s  = W @ (h / Z) + b                  # [16]  (sum of softmax = 1 absorbs bias)
        v  = squash_diag(s);  V += v
    out[b, k, :] = v  for all k

where squash_diag(s)[d] = s[d] * m / ((1+m)(1+m^2)), m = sqrt(10)*|s[d]|.

Mapping: batch on partitions (128 per bank, 4 banks/core), u in natural
layout [128, 1152*8] (fully contiguous DMA).  Streaming work per iteration:
the c-chain (8 fused MAC passes) runs on DVE via scalar_tensor_tensor with
per-partition scalars; exp+sum runs on ACT; the h-dots are split — DVE
scalar_tensor_tensor with accum for the first N_DVE_H columns, GPSIMD
tensor_tensor products + ACT accumulation passes for the rest.  Bank phases
are emitted in a skewed wavefront so the in-order engine queues pipeline
across banks.  No PE, no transposes, no cross-core communication.
"""

import sys

if "/opt/trn_rl_repo" not in sys.path:
    sys.path.insert(0, "/opt/trn_rl_repo")

import numpy as np
from contextlib import ExitStack

import concourse.bass as bass
import concourse.bacc as bacc
import concourse.tile as tile
from concourse import mybir
from concourse.bass_utils import run_bass_kernel_spmd

F32 = mybir.dt.float32
AF = mybir.ActivationFunctionType
OP = mybir.AluOpType

N_CORES = 8
B_FULL = 4096
B_CORE = B_FULL // N_CORES  # 512
N = 1152
DIN = 8
DOUT = 16
K = 10
BANKS = B_CORE // 128  # 4
SQ10 = float(np.sqrt(10.0))

N_DVE_H = 352          # h columns handled by DVE stt passes
GP_SLICES = 2          # GPSIMD h product slices (rest of the columns)
GP_COLS = (N - N_DVE_H) // GP_SLICES  # 400

_compiled = None


def _build():
    nc = bacc.Bacc("TRN2", target_bir_lowering=False, debug=False)
    u_d = nc.dram_tensor("u", [B_CORE, N, DIN], F32, kind="ExternalInput").ap()
    w_d = nc.dram_tensor("W", [DOUT, DIN], F32, kind="ExternalInput").ap()
    b_d = nc.dram_tensor("b", [1, DOUT], F32, kind="ExternalInput").ap()
    o_d = nc.dram_tensor("out", [B_CORE, K, DOUT], F32, kind="ExternalOutput").ap()

    with tile.TileContext(nc) as tc, ExitStack() as ctx:
        const = ctx.enter_context(tc.tile_pool(name="const", bufs=1))
        upool = ctx.enter_context(tc.tile_pool(name="u", bufs=3))
        big = ctx.enter_context(tc.tile_pool(name="big", bufs=2))
        scr1 = ctx.enter_context(tc.tile_pool(name="scr1", bufs=1))
        pdve = ctx.enter_context(tc.tile_pool(name="pdve", bufs=2))
        pascr = ctx.enter_context(tc.tile_pool(name="pascr", bufs=2))
        pgp = ctx.enter_context(tc.tile_pool(name="pgp", bufs=2))
        small = ctx.enter_context(tc.tile_pool(name="small", bufs=6))

        wrep = const.tile([128, DOUT * DIN], F32, tag="wrep")
        nc.sync.dma_start(wrep[:], w_d.flatten().unsqueeze(0).broadcast_to([128, DOUT * DIN]))
        brep = const.tile([128, DOUT], F32, tag="brep")
        nc.sync.dma_start(brep[:], b_d.broadcast_to([128, DOUT]))

        def small_matvec(vec8):
            """s[., d] = sum_i W[d, i] * vec8[., i] + b  -> [128, 16]."""
            prod = small.tile([128, DOUT * DIN], F32, tag="mv_prod")
            vb = vec8[:].unsqueeze(1).broadcast_to([128, DOUT, DIN])
            nc.vector.tensor_mul(
                prod[:].rearrange("p (d i) -> p d i", i=DIN),
                wrep[:].rearrange("p (d i) -> p d i", i=DIN), vb,
            )
            s_nob = small.tile([128, DOUT], F32, tag="mv_red")
            nc.vector.tensor_reduce(
                out=s_nob[:], in_=prod[:].rearrange("p (d i) -> p d i", i=DIN),
                axis=mybir.AxisListType.X, op=OP.add,
            )
            s = small.tile([128, DOUT], F32, tag="s_tile")
            nc.vector.tensor_add(s[:], s_nob[:], brep[:])
            return s

        def small_matvec_T(vec16):
            """g[., i] = sum_d W[d, i] * vec16[., d]  -> [128, 8]."""
            prod = small.tile([128, DOUT * DIN], F32, tag="mvt_prod")
            vb = vec16[:].unsqueeze(2).broadcast_to([128, DOUT, DIN])
            nc.vector.tensor_mul(
                prod[:].rearrange("p (d i) -> p d i", i=DIN),
                wrep[:].rearrange("p (d i) -> p d i", i=DIN), vb,
            )
            g = small.tile([128, DIN], F32, tag="g")
            nc.vector.tensor_reduce(
                out=g[:], in_=prod[:].rearrange("p (d i) -> p i d", i=DIN),
                axis=mybir.AxisListType.X, op=OP.add,
            )
            return g

        def squash(s):
            m = small.tile([128, DOUT], F32, tag="sq_m")
            nc.scalar.activation(out=m[:], in_=s[:], func=AF.Abs, scale=SQ10)
            m2 = small.tile([128, DOUT], F32, tag="sq_m2")
            nc.scalar.activation(out=m2[:], in_=s[:], func=AF.Square, scale=SQ10)
            d1 = small.tile([128, DOUT], F32, tag="sq_d1")
            nc.scalar.activation(out=d1[:], in_=m[:], func=AF.Identity, bias=1.0)
            den = small.tile([128, DOUT], F32, tag="sq_den")
            nc.vector.scalar_tensor_tensor(
                out=den[:], in0=m2[:], scalar=1.0, in1=d1[:], op0=OP.add, op1=OP.mult
            )
            rec = small.tile([128, DOUT], F32, tag="sq_rec")
            nc.vector.reciprocal(rec[:], den[:])
            sm = small.tile([128, DOUT], F32, tag="sq_sm")
            nc.vector.tensor_mul(sm[:], s[:], m[:])
            v = small.tile([128, DOUT], F32, tag="sq_v")
            nc.vector.tensor_mul(v[:], sm[:], rec[:])
            return v

        # per-bank live state across phases
        st = [dict() for _ in range(BANKS)]

        def ph_load(b):
            ub = upool.tile([128, N * DIN], F32, tag="ubank")
            src = u_d[b * 128 : (b + 1) * 128].rearrange("p n i -> p (n i)")
            nchunk = N * DIN // 4
            for q in range(4):
                nc.sync.dma_start(
                    ub[:, q * nchunk : (q + 1) * nchunk],
                    src[:, q * nchunk : (q + 1) * nchunk],
                )
            st[b]["uv"] = ub[:].rearrange("p (n i) -> p n i", i=DIN)

        def ph_init(b):
            uv = st[b]["uv"]
            r8 = small.tile([128, DIN], F32, tag="r8")
            for i in range(DIN):
                rscr = scr1.tile([128, N], F32, tag="rscr")
                nc.scalar.activation(
                    out=rscr[:], in_=uv[:, :, i], func=AF.Copy,
                    accum_out=r8[:, i : i + 1],
                )
            rN = small.tile([128, DIN], F32, tag="rN")
            nc.vector.tensor_scalar_mul(rN[:], r8[:], 1.0 / N)
            s = small_matvec(rN)
            v = squash(s)
            V = small.tile([128, DOUT], F32, tag="V")
            nc.vector.tensor_copy(V[:], v[:])
            st[b]["V"] = V

        def ph_logits(b):
            uv = st[b]["uv"]
            g = small_matvec_T(st[b]["V"])
            c_a = big.tile([128, N], F32, tag="c_a")
            c_b = big.tile([128, N], F32, tag="c_b")
            cur, nxt = c_a, c_b
            for i in range(DIN):
                if i == 0:
                    nc.vector.tensor_scalar_mul(cur[:], uv[:, :, i], g[:, 0:1])
                else:
                    nc.vector.scalar_tensor_tensor(
                        out=nxt[:], in0=uv[:, :, i], scalar=g[:, i : i + 1],
                        in1=cur[:], op0=OP.mult, op1=OP.add,
                    )
                    cur, nxt = nxt, cur
            E = big.tile([128, N], F32, tag="E")
            zp = small.tile([128, 2], F32, tag="zp")
            half = N // 2
            nc.scalar.activation(out=E[:, :half], in_=cur[:, :half], func=AF.Exp,
                                 accum_out=zp[:, 0:1])
            nc.scalar.activation(out=E[:, half:], in_=cur[:, half:], func=AF.Exp,
                                 accum_out=zp[:, 1:2])
            rz = small.tile([128, 1], F32, tag="rz")
            zs = small.tile([128, 1], F32, tag="zs")
            nc.vector.tensor_add(zs[:], zp[:, 0:1], zp[:, 1:2])
            nc.vector.reciprocal(rz[:], zs[:])
            st[b]["E"] = E
            st[b]["rz"] = rz

        def ph_update(b, last):
            uv = st[b]["uv"]
            E = st[b]["E"]
            rz = st[b]["rz"]
            # DVE columns [0, N_DVE_H): fused mac with accumulator output
            h_dve = small.tile([128, DIN], F32, tag="h_dve")
            for i in range(DIN):
                pscr = pdve.tile([128, N_DVE_H], F32, tag="pdve")
                nc.vector.scalar_tensor_tensor(
                    out=pscr[:], in0=uv[:, :N_DVE_H, i], scalar=1.0,
                    in1=E[:, :N_DVE_H], op0=OP.mult, op1=OP.mult,
                    accum_out=h_dve[:, i : i + 1],
                )
            # GPSIMD columns: products in i-major layout; ACT accumulates
            h_gps = []
            for sl in range(GP_SLICES):
                lo = N_DVE_H + sl * GP_COLS
                hi = lo + GP_COLS
                prod = pgp.tile([128, DIN * GP_COLS], F32, tag="pgp")
                pv = prod[:].rearrange("p (i n) -> p i n", i=DIN)
                uvi = uv[:, lo:hi, :].transpose([0, 2, 1])  # [128, i, n]
                eb = E[:, lo:hi].unsqueeze(1).broadcast_to([128, DIN, hi - lo])
                nc.gpsimd.tensor_mul(pv, uvi, eb)
                h_gp = small.tile([128, DIN], F32, tag=f"h_gp{sl}")
                for i in range(DIN):
                    ascr = pascr.tile([128, GP_COLS], F32, tag="ascr")
                    nc.scalar.activation(
                        out=ascr[:], in_=pv[:, i, :], func=AF.Copy,
                        accum_out=h_gp[:, i : i + 1],
                    )
                h_gps.append(h_gp)
            h8 = small.tile([128, DIN], F32, tag="h8")
            nc.vector.tensor_add(h8[:], h_dve[:], h_gps[0][:])
            for sl in range(1, GP_SLICES):
                h8n = small.tile([128, DIN], F32, tag="h8")
                nc.vector.tensor_add(h8n[:], h8[:], h_gps[sl][:])
                h8 = h8n
            su = small.tile([128, DIN], F32, tag="su")
            nc.vector.tensor_scalar_mul(su[:], h8[:], rz[:, 0:1])
            s = small_matvec(su)
            v = squash(s)
            if not last:
                V2 = small.tile([128, DOUT], F32, tag="V")
                nc.vector.tensor_add(V2[:], st[b]["V"], v[:])
                st[b]["V"] = V2
            else:
                orep = small.tile([128, K * DOUT], F32, tag="orep")
                nc.scalar.copy(
                    orep[:].rearrange("p (k d) -> p k d", k=K),
                    v[:].unsqueeze(1).broadcast_to([128, K, DOUT]),
                )
                nc.sync.dma_start(
                    o_d[b * 128 : (b + 1) * 128].rearrange("p k d -> p (k d)"),
                    orep[:],
                )

        phases = [
            ph_load,
            ph_init,
            lambda b: ph_logits(b),
            lambda b: ph_update(b, last=False),
            lambda b: ph_logits(b),
            lambda b: ph_update(b, last=True),
        ]
        NPH = len(phases)
        for w in range(NPH + BANKS - 1):
            for b in range(BANKS - 1, -1, -1):
                p = w - b
                if 0 <= p < NPH:
                    phases[p](b)

    nc.compile()
    return nc


def _get_compiled():
    global _compiled
    if _compiled is None:
        _compiled = _build()
    return _compiled


def kernel(u: np.ndarray, W: np.ndarray, b: np.ndarray) -> np.ndarray:
    nc = _get_compiled()
    u = np.ascontiguousarray(u, dtype=np.float32)
    W = np.ascontiguousarray(W, dtype=np.float32)
    b2 = np.ascontiguousarray(b, dtype=np.float32).reshape(1, DOUT)
    in_maps = [
        {"u": u[c * B_CORE : (c + 1) * B_CORE], "W": W, "b": b2}
        for c in range(N_CORES)
    ]
    res = run_bass_kernel_spmd(nc, in_maps, core_ids=list(range(N_CORES)))
    out = np.concatenate([res.results[c]["out"] for c in range(N_CORES)], axis=0)
    return out
